# revision 14
# baseline (speedup 1.0000x reference)
"""Tacotron2-style decoder on 8 Trainium2 NeuronCores.

Strategy (collectives measured at ~1ms each here -> pure data-parallel):
- batch 64 -> 8 per core, full T=800 recurrence per core, no cross-core comm.
- States kept transposed (feature-on-partitions, batch=8 on free dim) so the
  LSTM gate GEMMs run weight-stationary (lhsT = weight tile, rhs = [128,8]
  state chunk) and gate outputs land directly in gate-major layout for the
  elementwise LSTM update.
- attn-side weights pinned in SBUF (bf16), dec-side weights streamed from HBM
  each step, double-buffered, overlapped with PE.
- prenet, processed_memory, weight packing/casts, masks all precomputed on
  host in numpy (pure functions of the inputs).
"""
import numpy as np
import ml_dtypes

import concourse.bass as bass
import concourse.tile as tile
import concourse.mybir as mybir
from concourse.bass_utils import run_bass_kernel_spmd

BF16 = ml_dtypes.bfloat16
N_CORES = 8
B, S, ENC, T_FULL = 64, 256, 512, 800
BPC = B // N_CORES  # 8
NM, PRENET, ARNN, DRNN, ATTN = 80, 256, 1024, 1024, 128
LOC_F, LOC_K = 32, 31
KA = PRENET + ENC + ARNN      # 1792 -> 14 chunks
KD = ARNN + ENC + DRNN        # 2560 -> 20 chunks
NKA, NKD = KA // 128, KD // 128
NMT = 4 * ARNN // 128         # 32 M tiles of the 4096 gate dim
F32 = mybir.dt.float32
BF = mybir.dt.bfloat16

_BUILD_CACHE = {}


def _split_multi_waits(nc):
    """This container's walrus rejects >1 semaphore wait per instruction.
    Hoist extra waits onto same-engine NOPs inserted just before (engine
    queues run in order, so earlier waits on the same queue are equivalent)."""
    ctr = 0
    for f in nc.m.functions:
        for blk in f.blocks:
            insts = blk.instructions
            if not any(
                i.sync_info is not None and len(i.sync_info.on_wait) > 1
                for i in insts
            ):
                continue
            out = []
            for inst in insts:
                si = inst.sync_info
                if si is not None and len(si.on_wait) > 1:
                    waits = list(si.on_wait)
                    for w in waits[:-1]:
                        ctr += 1
                        out.append(
                            mybir.InstNoOp(
                                name=f"waitsplit-{ctr}",
                                engine=inst.engine,
                                bass_nofuse=True,
                                sync_info=mybir.SyncInfo(on_wait=[w], on_update=[]),
                            )
                        )
                    inst.sync_info = mybir.SyncInfo(
                        on_wait=[waits[-1]], on_update=list(si.on_update)
                    )
                out.append(inst)
            blk.instructions = out


def _build(T):
    nc = bass.Bass(num_devices=N_CORES)
    d = {}
    d["w_attn"] = nc.dram_tensor("w_attn", [NKA, 128, 4096], BF, kind="ExternalInput")
    d["w_dec"] = nc.dram_tensor("w_dec", [NKD, 128, 4096], BF, kind="ExternalInput")
    d["mem4"] = nc.dram_tensor("mem4", [128, BPC * 2 * ENC], BF, kind="ExternalInput")
    d["pmT"] = nc.dram_tensor("pmT", [128, BPC * S], F32, kind="ExternalInput")
    d["x2T"] = nc.dram_tensor("x2T", [T, 128, 2, BPC], F32, kind="ExternalInput")
    d["qwT"] = nc.dram_tensor("qwT", [128, 8 * 128], BF, kind="ExternalInput")
    d["fusedT"] = nc.dram_tensor("fusedT", [62, 128], F32, kind="ExternalInput")
    d["vT"] = nc.dram_tensor("vT", [128, 1], BF, kind="ExternalInput")
    d["projT"] = nc.dram_tensor("projT", [128, 12 * 81], BF, kind="ExternalInput")
    d["mask01"] = nc.dram_tensor("mask01", [BPC, S], F32, kind="ExternalInput")
    d["biasrep"] = nc.dram_tensor("biasrep", [BPC, 81], F32, kind="ExternalInput")
    d["ident"] = nc.dram_tensor("ident", [128, 128], F32, kind="ExternalInput")
    out_melg = nc.dram_tensor("out_melg", [T, BPC, 81], F32, kind="ExternalOutput")
    out_align = nc.dram_tensor("out_align", [T, BPC, S], F32, kind="ExternalOutput")

    AF = mybir.ActivationFunctionType
    AX = mybir.AxisListType.X

    with tile.TileContext(nc) as tc:
        with (
            tc.tile_pool(name="P1", bufs=1) as P1,
            tc.tile_pool(name="P2", bufs=2) as P2,
            tc.tile_pool(name="WS", bufs=2) as WS,
            tc.tile_pool(name="P3", bufs=1) as P3,
            tc.tile_pool(name="DR", bufs=1, space="DRAM") as DR,
            tc.tile_pool(name="ps1", bufs=2, space="PSUM") as ps1,
            tc.tile_pool(name="ps2", bufs=2, space="PSUM") as ps2,
            tc.tile_pool(name="ps3", bufs=3, space="PSUM") as ps3,
            tc.tile_pool(name="ps4", bufs=1, space="PSUM") as ps4,
        ):
            # ---- resident tensors ----
            NPIN = NKA - 4
            wA = P1.tile([128, NPIN * 4096], BF, tag="wA")
            for k in range(NPIN):
                nc.sync.dma_start(
                    wA[:, k * 4096:(k + 1) * 4096], d["w_attn"][k, :, :]
                )
            mem_sb = P1.tile([128, BPC * 2 * ENC], BF, tag="mem")
            nc.sync.dma_start(mem_sb[:], d["mem4"][:, :])
            pmT = P1.tile([128, BPC * S], F32, tag="pmT")
            nc.sync.dma_start(pmT[:], d["pmT"][:, :])
            qwT = P1.tile([128, 8 * 128], BF, tag="qwT")
            nc.sync.dma_start(qwT[:], d["qwT"][:, :])
            fusedT = P1.tile([62, 128], F32, tag="fusedT")
            nc.sync.dma_start(fusedT[:], d["fusedT"][:, :])
            vT = P1.tile([128, 1], BF, tag="vT")
            nc.sync.dma_start(vT[:], d["vT"][:, :])
            projT = P1.tile([128, 12 * 81], BF, tag="projT")
            nc.sync.dma_start(projT[:], d["projT"][:, :])
            mask = P1.tile([BPC, S], F32, tag="mask")
            nc.sync.dma_start(mask[:], d["mask01"][:, :])
            bias = P1.tile([BPC, 81], F32, tag="bias")
            nc.sync.dma_start(bias[:], d["biasrep"][:, :])
            ident = P1.tile([128, 128], F32, tag="ident")
            nc.sync.dma_start(ident[:], d["ident"][:, :])

            # ---- states ----
            ahT = P1.tile([128, 64], F32, tag="ahT")
            acT = P1.tile([128, 64], F32, tag="acT")
            dhT = P1.tile([128, 64], F32, tag="dhT")
            dcT = P1.tile([128, 64], F32, tag="dcT")
            ahTb = P1.tile([128, 64], BF, tag="ahTb")
            dhTb = P1.tile([128, 64], BF, tag="dhTb")
            ctxTb = P1.tile([128, 32], BF, tag="ctxTb")
            aw = P1.tile([BPC, S], F32, tag="aw")
            awc = P1.tile([BPC, S], F32, tag="awc")
            catP = DR.tile([2, BPC, 286], F32, tag="catP")
            winT = P1.tile([62, BPC * S], F32, tag="winT")
            catZ = P1.tile([1, 2 * BPC * 286], F32, tag="catZ")
            for st in (ahT, acT, dhT, dcT, ahTb, dhTb, ctxTb, aw, awc, catZ, winT):
                nc.vector.memset(st[:], 0.0)
            nc.sync.dma_start(
                catP[:, :, :].rearrange("c b s -> (c b s)"), catZ[0:1, :]
            )

            def lstm_elem(g, cT, hT, hTb):
                si = P2.tile([128, 64], F32, tag="si")
                sf = P2.tile([128, 64], F32, tag="sf")
                tg = P2.tile([128, 64], F32, tag="tg")
                so = P2.tile([128, 64], F32, tag="so")
                nc.scalar.activation(si[:], g[:, 0:64], AF.Sigmoid)
                nc.scalar.activation(sf[:], g[:, 64:128], AF.Sigmoid)
                nc.scalar.activation(tg[:], g[:, 128:192], AF.Tanh)
                nc.scalar.activation(so[:], g[:, 192:256], AF.Sigmoid)
                t1 = P2.tile([128, 64], F32, tag="t1")
                t2 = P2.tile([128, 64], F32, tag="t2")
                nc.vector.tensor_mul(t1[:], sf[:], cT[:])
                nc.vector.tensor_mul(t2[:], si[:], tg[:])
                nc.vector.tensor_add(cT[:], t1[:], t2[:])
                tcn = P2.tile([128, 64], F32, tag="tcn")
                nc.scalar.activation(tcn[:], cT[:], AF.Tanh)
                nc.vector.tensor_mul(hT[:], so[:], tcn[:])
                nc.vector.tensor_copy(hTb[:], hT[:])

            with tc.For_i(0, T, 1) as iv:
                # prenet x_t (transposed) load + bf16 cast
                xb = P2.tile([128, 16], F32, tag="xb")
                nc.sync.dma_start(
                    xb[:].rearrange("p (c b) -> p c b", c=2),
                    d["x2T"][bass.ds(iv, 1), :, :, :],
                )
                xbb = P2.tile([128, 16], BF, tag="xbb")
                nc.vector.tensor_copy(xbb[:], xb[:])

                a_rhs = (
                    [xbb[:, c * 8:(c + 1) * 8] for c in range(2)]
                    + [ctxTb[:, c * 8:(c + 1) * 8] for c in range(4)]
                    + [ahTb[:, c * 8:(c + 1) * 8] for c in range(8)]
                )
                gA = ps1.tile([128, 256], F32, tag="ge")
                for k in range(NKA):
                    if k < NPIN:
                        slab = wA[:, k * 4096:(k + 1) * 4096]
                    else:
                        wsa = WS.tile([128, 4096], BF, tag="wdec")
                        nc.sync.dma_start(wsa[:], d["w_attn"][k, :, :])
                        slab = wsa[:]
                    for m in range(NMT):
                        nc.tensor.matmul(
                            gA[:, m * 8:(m + 1) * 8],
                            lhsT=slab[:, m * 128:(m + 1) * 128],
                            rhs=a_rhs[k],
                            start=(k == 0),
                            stop=(k == NKA - 1),
                        )
                lstm_elem(gA, acT, ahT, ahTb)

                # q = qw @ ah  -> [128a, 8b] psum
                qp = ps3.tile([128, 2048 // 4], F32, tag="sm")
                for k in range(8):
                    nc.tensor.matmul(
                        qp[:, 0:8],
                        lhsT=qwT[:, k * 128:(k + 1) * 128],
                        rhs=ahTb[:, k * 8:(k + 1) * 8],
                        start=(k == 0),
                        stop=(k == 7),
                    )

                # location features: plT = fused.T.T @ winT, in 4 quarters
                X2 = P3.tile([128, BPC * S], F32, tag="X2")
                for j in range(4):
                    pl = ps4.tile([128, 512], F32, tag="pl")
                    nc.tensor.matmul(
                        pl[:],
                        lhsT=fusedT[:],
                        rhs=winT[:, j * 512:(j + 1) * 512],
                        start=True,
                        stop=True,
                    )
                    nc.vector.tensor_add(
                        X2[:, j * 512:(j + 1) * 512], pl[:], pmT[:, j * 512:(j + 1) * 512]
                    )
                # + q broadcast over s (per-batch column add)
                tanhX = P3.tile([128, BPC * S], BF, tag="tanhX")
                for b in range(BPC):
                    nc.vector.tensor_scalar_add(
                        X2[:, b * S:(b + 1) * S],
                        X2[:, b * S:(b + 1) * S],
                        qp[:, b:b + 1],
                    )
                nc.scalar.activation(tanhX[:], X2[:], AF.Tanh)

                # energies -> [8, 256] psum, then masked exp and softmax pieces
                e8 = P3.tile([1, BPC * S], F32, tag="e8")
                for b in range(BPC):
                    ep = ps1.tile([128, 256], F32, tag="ge")
                    nc.tensor.matmul(
                        ep[0:1, :],
                        lhsT=vT[:],
                        rhs=tanhX[:, b * S:(b + 1) * S],
                        start=True,
                        stop=True,
                    )
                    nc.vector.tensor_copy(e8[0:1, b * S:(b + 1) * S], ep[0:1, :])
                e_sb = P3.tile([BPC, S], F32, tag="e_sb")
                nc.sync.dma_start(
                    e_sb[:, :], e8[0:1, :].rearrange("p (b s) -> (p b) s", b=BPC)
                )
                expm = P3.tile([BPC, S], F32, tag="expm")
                nc.scalar.activation(expm[:], e_sb[:], AF.Exp)
                nc.vector.tensor_mul(expm[:], expm[:], mask[:])
                Z = P2.tile([BPC, 1], F32, tag="Z")
                nc.vector.reduce_sum(Z[:], expm[:], axis=AX)
                rZ = P2.tile([BPC, 1], F32, tag="rZ")
                nc.vector.reciprocal(rZ[:], Z[:])
                nc.vector.tensor_scalar_mul(aw[:], expm[:], rZ[:])
                nc.vector.tensor_add(awc[:], awc[:], aw[:])
                nc.sync.dma_start(out_align[bass.ds(iv, 1), :, :], aw[:])

                # expT (bf16) for the context matvecs
                expTb = P2.tile([128, 16], BF, tag="expTb")
                for c in range(2):
                    tp = ps3.tile([128, 512], F32, tag="sm")
                    nc.tensor.transpose(
                        tp[:, 0:8], expm[:, c * 128:(c + 1) * 128], ident[0:8, 0:8]
                    )
                    nc.vector.tensor_copy(expTb[:, c * 8:(c + 1) * 8], tp[:, 0:8])
                # ctxU[b,:] = exp_b @ mem_b  (K=256 split in 2)
                c8 = P3.tile([1, BPC * ENC], F32, tag="c8")
                for b in range(BPC):
                    cu = ps3.tile([128, 512], F32, tag="sm")
                    for c in range(2):
                        nc.tensor.matmul(
                            cu[0:1, :],
                            lhsT=expTb[:, c * 8 + b:c * 8 + b + 1],
                            rhs=mem_sb[:, (b * 2 + c) * ENC:(b * 2 + c + 1) * ENC],
                            start=(c == 0),
                            stop=(c == 1),
                        )
                    nc.vector.tensor_copy(c8[0:1, b * ENC:(b + 1) * ENC], cu[0:1, :])
                cu_sb = P3.tile([BPC, ENC], F32, tag="cu_sb")
                nc.sync.dma_start(
                    cu_sb[:, :], c8[0:1, :].rearrange("p (b e) -> (p b) e", b=BPC)
                )
                ctxb = P3.tile([BPC, ENC], F32, tag="ctxb")
                nc.vector.tensor_scalar_mul(ctxb[:], cu_sb[:], rZ[:])
                for c in range(4):
                    tp2 = ps3.tile([128, 512], F32, tag="sm")
                    nc.tensor.transpose(
                        tp2[:, 0:8], ctxb[:, c * 128:(c + 1) * 128], ident[0:8, 0:8]
                    )
                    nc.vector.tensor_copy(ctxTb[:, c * 8:(c + 1) * 8], tp2[:, 0:8])

                # dec gates: stream weight slabs from HBM
                d_rhs = (
                    [ahTb[:, c * 8:(c + 1) * 8] for c in range(8)]
                    + [ctxTb[:, c * 8:(c + 1) * 8] for c in range(4)]
                    + [dhTb[:, c * 8:(c + 1) * 8] for c in range(8)]
                )
                gD = ps2.tile([128, 256], F32, tag="dp")
                for k in range(NKD):
                    ws = WS.tile([128, 4096], BF, tag="wdec")
                    nc.sync.dma_start(ws[:], d["w_dec"][k, :, :])
                    for m in range(NMT):
                        nc.tensor.matmul(
                            gD[:, m * 8:(m + 1) * 8],
                            lhsT=ws[:, m * 128:(m + 1) * 128],
                            rhs=d_rhs[k],
                            start=(k == 0),
                            stop=(k == NKD - 1),
                        )
                lstm_elem(gD, dcT, dhT, dhTb)

                # proj + gate
                hc_chunks = [dhTb[:, c * 8:(c + 1) * 8] for c in range(8)] + [
                    ctxTb[:, c * 8:(c + 1) * 8] for c in range(4)
                ]
                pj = ps2.tile([BPC, 256], F32, tag="dp")
                for k in range(12):
                    nc.tensor.matmul(
                        pj[:, 0:81],
                        lhsT=hc_chunks[k],
                        rhs=projT[:, k * 81:(k + 1) * 81],
                        start=(k == 0),
                        stop=(k == 11),
                    )
                melg = P2.tile([BPC, 81], F32, tag="melg")
                nc.vector.tensor_add(melg[:], pj[:, 0:81], bias[:])
                nc.sync.dma_start(out_melg[bass.ds(iv, 1), :, :], melg[:])

                # build winT for next step: aw/awc -> padded cat -> 62 windows
                import bass_rust as _br
                for c, s_t in ((0, aw), (1, awc)):
                    nc.sync.dma_start(catP[c, :, 15:271], s_t[:, :])
                for c in range(2):
                    src_ap = catP[c, 0:1, 0:1]
                    src_ap.ap = _br.VecI64Pair([[1, 31], [286, BPC], [1, 256]])
                    dst = winT[c * 31:(c + 1) * 31, 0:1]
                    dst.ap = _br.VecI64Pair(
                        [list(dst.ap[0]), [256, BPC], [1, 256]]
                    )
                    nc.sync.dma_start(dst, src_ap)

    _split_multi_waits(nc)
    return nc


def _host_pack(inputs, T):
    """Numpy precompute of everything derivable from the inputs."""
    g = lambda k: np.asarray(inputs[k], np.float32)
    mem = g("memory")
    dec_in = g("decoder_inputs")[:, :, :T]
    mlen = np.asarray(inputs["memory_lengths"])

    # prenet over all timesteps (teacher forcing: input-independent of state)
    Dn = np.transpose(dec_in, (2, 0, 1))  # [T,B,80]
    x = np.concatenate([np.zeros((1, B, NM), np.float32), Dn[:-1]], 0)
    x = np.maximum(x @ g("prenet_w1").T, 0.0)
    x = np.maximum(x @ g("prenet_w2").T, 0.0)  # [T,B,256]

    pm = mem @ g("memory_w").T  # [B,S,128]

    w_attn = np.concatenate([g("attn_rnn_wih").T, g("attn_rnn_whh").T], 0)
    w_attn = w_attn.reshape(NKA, 128, 4096).astype(BF16)
    w_dec = np.concatenate([g("dec_rnn_wih").T, g("dec_rnn_whh").T], 0)
    w_dec = w_dec.reshape(NKD, 128, 4096).astype(BF16)
    qwT = np.ascontiguousarray(g("query_w").T)  # [1024,128]
    qwT_packed = np.ascontiguousarray(
        qwT.reshape(8, 128, 128).transpose(1, 0, 2).reshape(128, 8 * 128)
    ).astype(BF16)
    fused = np.einsum("af,fck->ack", g("loc_dense_w"), g("loc_conv_w"))  # [128,2,31]
    fusedT = fused.reshape(128, 62).T.astype(np.float32)  # [62,128]
    vT = g("v_w").T.astype(BF16)  # [128,1]
    projW = np.concatenate([g("proj_w"), g("gate_w")], 0)  # [81,1536]
    projT = projW.T.reshape(12, 128, 81).transpose(1, 0, 2).reshape(128, 12 * 81)
    projT = np.ascontiguousarray(projT).astype(BF16)
    biasfull = np.concatenate([g("proj_b"), g("gate_b")])  # [81]
    ident = np.eye(128, dtype=np.float32)

    in_maps = []
    for c in range(N_CORES):
        sl = slice(c * BPC, (c + 1) * BPC)
        mem_c = mem[sl]  # [8,256,512]
        mem4 = (
            mem_c.reshape(BPC, 2, 128, ENC).transpose(2, 0, 1, 3).reshape(128, -1)
        )
        pmT_c = pm[sl].transpose(2, 0, 1).reshape(128, BPC * S)
        x_c = x[:, sl, :]  # [T,8,256]
        x2T = (
            x_c.transpose(0, 2, 1).reshape(T, 2, 128, BPC).transpose(0, 2, 1, 3)
        )
        mask_c = (np.arange(S)[None, :] < mlen[sl, None]).astype(np.float32)
        in_maps.append(
            {
                "w_attn": w_attn,
                "w_dec": w_dec,
                "mem4": np.ascontiguousarray(mem4).astype(BF16),
                "pmT": np.ascontiguousarray(pmT_c).astype(np.float32),
                "x2T": np.ascontiguousarray(x2T).astype(np.float32),
                "qwT": np.ascontiguousarray(qwT_packed).astype(BF16),
                "fusedT": np.ascontiguousarray(fusedT),
                "vT": np.ascontiguousarray(vT),
                "projT": projT,
                "mask01": mask_c,
                "biasrep": np.tile(biasfull[None, :], (BPC, 1)).astype(np.float32),
                "ident": ident,
            }
        )
    return in_maps


def kernel(**inputs):
    T = int(np.asarray(inputs["decoder_inputs"]).shape[2])
    if T not in _BUILD_CACHE:
        _BUILD_CACHE[T] = _build(T)
    nc = _BUILD_CACHE[T]
    in_maps = _host_pack(inputs, T)
    res = run_bass_kernel_spmd(nc, in_maps, list(range(N_CORES))).results

    mel = np.zeros((B, NM, T), np.float32)
    gate = np.zeros((B, T), np.float32)
    align = np.zeros((B, T, S), np.float32)
    for c in range(N_CORES):
        sl = slice(c * BPC, (c + 1) * BPC)
        mg = res[c]["out_melg"]  # [T,8,81]
        mel[sl] = mg[:, :, :NM].transpose(1, 2, 0)
        gate[sl] = mg[:, :, NM].T
        align[sl] = res[c]["out_align"].transpose(1, 0, 2)
    return mel, gate, align
